# revision 10
# baseline (speedup 1.0000x reference)
"""2-layer GAT (GATConv x2, PyG-style) on 8 Trainium2 NeuronCores.

Strategy (edge-parallel, dst-sharded):
  - Nodes padded to NP = 8*98*64 = 50176 and sharded contiguously: core c
    owns nodes [c*6272, (c+1)*6272), i.e. 98 windows of W=64 dst nodes.
  - Edges (incl. self loops) are sorted by dst window on the host; each core
    processes exactly the edges that land in its dst windows, so no
    cross-core reduction of messages is needed.
  - Node phase: each core computes rows [h | s_src | s_dst] = x @ Wcat for
    its node slice, then an AllGather builds the full gather table in DRAM.
  - Edge phase: per 64-dst-node window, edges are processed in blocks of
    128 (one edge per partition).  Indirect DMA gathers [h|s_src] rows by
    src id and s_dst by dst id.  Scores e = leakyrelu(sS+sD), p = exp(e)
    (no segment-max needed: scores are bounded, exp stays in f32 range).
    A one-hot selection matrix (is_equal vs iota) + PE matmul accumulates
    both the denominator sum(p) and the messages sum(p * h_src) into PSUM
    per dst slot; the softmax division happens once per dst row at drain.
  - Per-core window->slot assignment is sorted by edge count so all cores
    share one SPMD program (slot block counts = max over cores of the
    order statistics).  The resulting per-core node permutation is folded
    into the layer-2 gather indices; the host un-permutes the output.
"""

import numpy as np

P = 128          # edges per block / SBUF partitions
W = 64           # dst nodes per window
NC = 8           # cores
WPC = 98         # windows per core
NPC = WPC * W    # nodes per core (6272)
NP = NC * NPC    # padded node count (50176)
IN_DIM = 128
HEADS1, HID1 = 8, 8
OUT_DIM = 64
NEG_SLOPE = 0.2
SUPER_BLK = 72   # max gather blocks per indirect-DMA super instruction


def _mk_head_mat(a):
    """[H, C] attention vector -> [H*C, H] block-diagonal matrix."""
    H, C = a.shape
    A = np.zeros((H * C, H), np.float32)
    for h in range(H):
        A[h * C:(h + 1) * C, h] = a[h]
    return A


def _prep(x, edge_index, W1, a_src1, a_dst1, b1, W2, a_src2, a_dst2, b2,
          n_cores=NC, wpc=WPC):
    """Host-side preprocessing. Returns (cfg, in_maps, perm)."""
    npc = wpc * W
    n_pad = n_cores * npc
    n = x.shape[0]
    assert n <= n_pad

    x = np.asarray(x, np.float32)
    xp = np.zeros((n_pad, IN_DIM), np.float32)
    xp[:n] = x

    ei = np.asarray(edge_index)
    src = np.concatenate([ei[0], np.arange(n)]).astype(np.int64)
    dst = np.concatenate([ei[1], np.arange(n)]).astype(np.int64)

    # sort edges by destination window
    win = (dst // W).astype(np.int64)
    order = np.argsort(win, kind="stable")
    src, dst, win = src[order], dst[order], win[order]
    nw = n_pad // W
    counts = np.bincount(win, minlength=nw)
    starts = np.concatenate([[0], np.cumsum(counts)])

    counts_c = counts.reshape(n_cores, wpc)
    K_c = np.ceil(counts_c / P).astype(np.int64)          # blocks per window
    orders = [np.argsort(-counts_c[c], kind="stable") for c in range(n_cores)]
    Ks = np.max(np.stack([K_c[c][orders[c]] for c in range(n_cores)]), axis=0)
    Ks = np.maximum(Ks, 1)  # keep every slot non-degenerate
    Mtot = int(Ks.sum())

    # perm[node] = global row in the slot-ordered (layer-2) table
    perm = np.empty(n_pad, np.int64)
    for c in range(n_cores):
        inv = np.empty(wpc, np.int64)
        inv[orders[c]] = np.arange(wpc)
        wl = np.arange(wpc)
        base = (c * wpc + wl) * W
        for woff in range(W):
            perm[base + woff] = c * npc + inv * W + woff

    def pack(arrs, dtype):
        # per-slot flat arrays -> [128, Mtot] with edge j*128+p at [p, j]
        cols = [a.reshape(-1, P).T for a in arrs]
        return np.ascontiguousarray(np.concatenate(cols, axis=1), dtype)

    in_maps = []
    for c in range(n_cores):
        esrc, edstg, esrc2, edst2, edstl = [], [], [], [], []
        for s in range(wpc):
            wloc = orders[c][s]
            wglob = c * wpc + wloc
            e0, e1 = starts[wglob], starts[wglob + 1]
            nslots = int(Ks[s]) * P
            npad = nslots - (e1 - e0)
            sw = src[e0:e1]
            dw = dst[e0:e1]
            z = np.zeros(npad, np.int64)
            esrc.append(np.concatenate([sw, z]))
            edstg.append(np.concatenate([dw, z]))
            esrc2.append(np.concatenate([perm[sw], z]))
            edst2.append(np.concatenate([perm[dw], z]))
            edstl.append(np.concatenate([dw - wglob * W,
                                         np.full(npad, W, np.int64)]))
        in_maps.append({
            "x": np.ascontiguousarray(xp[c * npc:(c + 1) * npc]),
            "esrc": pack(esrc, np.int32),
            "edst": pack(edstg, np.int32),
            "esrc2": pack(esrc2, np.int32),
            "edst2": pack(edst2, np.int32),
            "edstl": pack(edstl, np.float32),
        })

    W1 = np.asarray(W1, np.float32)
    W2 = np.asarray(W2, np.float32)
    wc1 = np.concatenate([W1, W1 @ _mk_head_mat(np.asarray(a_src1, np.float32)),
                          W1 @ _mk_head_mat(np.asarray(a_dst1, np.float32))],
                         axis=1)                     # [128, 80]
    wc2 = np.concatenate([W2, W2 @ np.asarray(a_src2, np.float32).T,
                          W2 @ np.asarray(a_dst2, np.float32).T], axis=1)  # [64, 66]
    b1r = np.tile(np.asarray(b1, np.float32)[None, :], (W, 1))
    b2r = np.tile(np.asarray(b2, np.float32)[None, :], (W, 1))
    for m in in_maps:
        m["wc1"] = np.ascontiguousarray(wc1, np.float32)
        m["wc2"] = np.ascontiguousarray(wc2, np.float32)
        m["b1r"] = np.ascontiguousarray(b1r, np.float32)
        m["b2r"] = np.ascontiguousarray(b2r, np.float32)

    cfg = dict(n_cores=n_cores, wpc=wpc, npc=npc, n_pad=n_pad,
               Ks=[int(k) for k in Ks], Mtot=Mtot)
    return cfg, in_maps, perm


def _sub(apbase, off, dims):
    """Custom multi-level free-dim AP on top of a tile's [:, :] AP."""
    import concourse.bass as bass
    return bass.AP(tensor=apbase.tensor, offset=apbase.offset + off,
                   ap=[list(apbase.ap[0])] + [list(d) for d in dims])


def _build(nc, cfg, debug_tabs=False):
    """Emit the full SPMD program into nc. Returns nothing."""
    import concourse.bass as bass
    import concourse.mybir as mybir
    import concourse.tile as tile
    from concourse.bass import IndirectOffsetOnAxis

    f32 = mybir.dt.float32
    i32 = mybir.dt.int32
    Alu = mybir.AluOpType
    Act = mybir.ActivationFunctionType

    n_cores, wpc, npc, n_pad = cfg["n_cores"], cfg["wpc"], cfg["npc"], cfg["n_pad"]
    Ks, Mtot = cfg["Ks"], cfg["Mtot"]
    groups = [list(range(n_cores))]

    # --- dram I/O ---
    x_d = nc.dram_tensor("x", [npc, IN_DIM], f32, kind="ExternalInput")
    esrc_d = nc.dram_tensor("esrc", [P, Mtot], i32, kind="ExternalInput")
    edst_d = nc.dram_tensor("edst", [P, Mtot], i32, kind="ExternalInput")
    esrc2_d = nc.dram_tensor("esrc2", [P, Mtot], i32, kind="ExternalInput")
    edst2_d = nc.dram_tensor("edst2", [P, Mtot], i32, kind="ExternalInput")
    edstl_d = nc.dram_tensor("edstl", [P, Mtot], f32, kind="ExternalInput")
    wc1_d = nc.dram_tensor("wc1", [IN_DIM, 80], f32, kind="ExternalInput")
    wc2_d = nc.dram_tensor("wc2", [64, 66], f32, kind="ExternalInput")
    b1r_d = nc.dram_tensor("b1r", [W, 64], f32, kind="ExternalInput")
    b2r_d = nc.dram_tensor("b2r", [W, 64], f32, kind="ExternalInput")
    out_d = nc.dram_tensor("out", [npc, OUT_DIM], f32, kind="ExternalOutput")

    shared = "Local"
    t1s_d = nc.dram_tensor("t1slice", [npc, 80], f32, kind="Internal")
    table1 = nc.dram_tensor("table1", [n_pad, 80], f32, kind="Internal",
                            addr_space=shared)
    t2s_d = nc.dram_tensor("t2slice", [npc, 66], f32, kind="Internal")
    table2 = nc.dram_tensor("table2", [n_pad, 66], f32, kind="Internal",
                            addr_space=shared)

    if debug_tabs:
        dbg1_d = nc.dram_tensor("dbg1", [n_pad, 80], f32, kind="ExternalOutput")
        dbg2_d = nc.dram_tensor("dbg2", [n_pad, 66], f32, kind="ExternalOutput")

    ident_d = nc.inline_tensor(np.eye(P, dtype=np.float32), "ident")
    iota_d = nc.inline_tensor(
        np.tile(np.arange(W, dtype=np.float32), (P, 1)), "iotaw")

    # supers: greedy grouping of slots by block budget
    supers = []  # list of (slot_start, nslots, blk_start, nblk)
    s0, b0 = 0, 0
    s = 0
    while s < wpc:
        nb = 0
        s0 = s
        while s < wpc and nb + Ks[s] <= SUPER_BLK:
            nb += Ks[s]
            s += 1
        supers.append((s0, s - s0, b0, nb))
        b0 += nb
    assert b0 == Mtot

    nt = npc // P  # node tiles per core

    with tile.TileContext(nc) as tc:
        with tc.tile_pool(name="const", bufs=1) as cp, \
             tc.tile_pool(name="work", bufs=3) as wp, \
             tc.tile_pool(name="gath", bufs=2) as gp, \
             tc.tile_pool(name="drain", bufs=3) as dp, \
             tc.tile_pool(name="eps", bufs=4, space="PSUM") as pp, \
             tc.tile_pool(name="nps", bufs=2, space="PSUM") as np_:

            ident = cp.tile([P, P], f32, tag="ident")
            nc.sync.dma_start(out=ident[:, :], in_=ident_d[:, :])
            iota = cp.tile([P, W], f32, tag="iota")
            nc.sync.dma_start(out=iota[:, :], in_=iota_d[:, :])
            wc1 = cp.tile([IN_DIM, 80], f32, tag="wc1")
            nc.sync.dma_start(out=wc1[:, :], in_=wc1_d[:, :])
            wc2 = cp.tile([64, 66], f32, tag="wc2")
            nc.sync.dma_start(out=wc2[:, :], in_=wc2_d[:, :])
            b1r = cp.tile([W, 64], f32, tag="b1r")
            nc.sync.dma_start(out=b1r[:, :], in_=b1r_d[:, :])
            b2r = cp.tile([W, 64], f32, tag="b2r")
            nc.sync.dma_start(out=b2r[:, :], in_=b2r_d[:, :])

            esrc = cp.tile([P, Mtot], i32, tag="esrc")
            nc.sync.dma_start(out=esrc[:, :], in_=esrc_d[:, :])
            edst = cp.tile([P, Mtot], i32, tag="edst")
            nc.sync.dma_start(out=edst[:, :], in_=edst_d[:, :])
            esrc2 = cp.tile([P, Mtot], i32, tag="esrc2")
            nc.sync.dma_start(out=esrc2[:, :], in_=esrc2_d[:, :])
            edst2 = cp.tile([P, Mtot], i32, tag="edst2")
            nc.sync.dma_start(out=edst2[:, :], in_=edst2_d[:, :])
            edstl = cp.tile([P, Mtot], f32, tag="edstl")
            nc.sync.dma_start(out=edstl[:, :], in_=edstl_d[:, :])

            h2big = cp.tile([P, (wpc // 2) * W], f32, tag="h2big")

            # ---------- node phase, layer 1 ----------
            for t in range(nt):
                xt = wp.tile([P, IN_DIM], f32, tag="xt")
                nc.sync.dma_start(out=xt[:, :], in_=x_d[t * P:(t + 1) * P, :])
                tp = np_.tile([IN_DIM, P], f32, tag="tps")
                nc.tensor.transpose(tp[:, :], xt[:, :], ident[:, :])
                xT = wp.tile([IN_DIM, P], f32, tag="xT")
                nc.vector.tensor_copy(out=xT[:, :], in_=tp[:, :])
                hp = np_.tile([P, 80], f32, tag="hps")
                nc.tensor.matmul(out=hp[:, :], lhsT=xT[:, :], rhs=wc1[:, :],
                                 start=True, stop=True)
                ht = wp.tile([P, 80], f32, tag="ht")
                nc.vector.tensor_copy(out=ht[:, :], in_=hp[:, :])
                nc.sync.dma_start(out=t1s_d[t * P:(t + 1) * P, :], in_=ht[:, :])

            nc.gpsimd.collective_compute(
                "AllGather", Alu.bypass, replica_groups=groups,
                ins=[t1s_d[:, :]], outs=[table1[:, :]])
            if debug_tabs:
                for t in range(n_pad // P):
                    dt_ = wp.tile([P, 80], f32, tag="dbg")
                    nc.sync.dma_start(out=dt_[:, :], in_=table1[t*P:(t+1)*P, :])
                    nc.sync.dma_start(out=dbg1_d[t*P:(t+1)*P, :], in_=dt_[:, :])

            # ---------- edge phases ----------
            def edge_phase(table, RL, GW, H, src_t, dstg_t, layer):
                SO = 64          # score col offset within gathered row
                for (sl0, nsl, bb0, nblk) in supers:
                    G = gp.tile([P, nblk * GW], f32, tag="G")
                    sD = gp.tile([P, nblk * H], f32, tag="sD")
                    for j in range(nblk):
                        nc.gpsimd.indirect_dma_start(
                            out=G[:, j * GW:(j + 1) * GW], out_offset=None,
                            in_=table[:, :],
                            in_offset=IndirectOffsetOnAxis(
                                ap=src_t[:, bb0 + j:bb0 + j + 1], axis=0))
                        nc.gpsimd.indirect_dma_start(
                            out=sD[:, j * H:(j + 1) * H], out_offset=None,
                            in_=table[:, :],
                            in_offset=IndirectOffsetOnAxis(
                                ap=dstg_t[:, bb0 + j:bb0 + j + 1], axis=0),
                            element_offset=64 + H)
                    # e = sS + sD ; lrelu ; p = exp -> back into G score cols
                    e = wp.tile([P, nblk * H], f32, tag="e")
                    nc.vector.tensor_tensor(
                        out=_sub(e[:, :], 0, [[H, nblk], [1, H]]),
                        in0=_sub(G[:, :], SO, [[GW, nblk], [1, H]]),
                        in1=_sub(sD[:, :], 0, [[H, nblk], [1, H]]),
                        op=Alu.add)
                    t02 = wp.tile([P, nblk * H], f32, tag="t02")
                    nc.vector.tensor_scalar_mul(t02[:, :], e[:, :], NEG_SLOPE)
                    nc.vector.tensor_tensor(out=e[:, :], in0=e[:, :],
                                            in1=t02[:, :], op=Alu.max)
                    nc.scalar.activation(
                        out=_sub(G[:, :], SO, [[GW, nblk], [1, H]]),
                        in_=_sub(e[:, :], 0, [[H, nblk], [1, H]]),
                        func=Act.Exp)
                    # onehot[e, d] = (dstl[e] == d)
                    oh = gp.tile([P, nblk * W], f32, tag="oh")
                    nc.vector.tensor_tensor(
                        out=_sub(oh[:, :], 0, [[W, nblk], [1, W]]),
                        in0=_sub(iota[:, :], 0, [[0, nblk], [1, W]]),
                        in1=_sub(edstl[:, :], bb0, [[1, nblk], [0, W]]),
                        op=Alu.is_equal)
                    # msg = h * p (per-head broadcast), in place on G h-cols
                    if H == 1:
                        in1p = _sub(G[:, :], SO, [[GW, nblk], [1, 1], [0, 64]])
                        in0m = _sub(G[:, :], 0, [[GW, nblk], [64, 1], [1, 64]])
                    else:
                        in1p = _sub(G[:, :], SO, [[GW, nblk], [1, H], [0, 64 // H]])
                        in0m = _sub(G[:, :], 0, [[GW, nblk], [64 // H, H], [1, 64 // H]])
                    nc.vector.tensor_tensor(out=in0m, in0=in0m, in1=in1p,
                                            op=Alu.mult)
                    # per-slot scatter matmuls + drain
                    bb = bb0
                    for s in range(sl0, sl0 + nsl):
                        K = Ks[s]
                        ps = pp.tile([W, GW], f32, tag="ps")
                        for j in range(K):
                            jj = bb - bb0 + j
                            nc.tensor.matmul(
                                out=ps[:, :],
                                lhsT=oh[:, jj * W:(jj + 1) * W],
                                rhs=G[:, jj * GW:(jj + 1) * GW],
                                start=(j == 0), stop=(j == K - 1))
                        bb += K
                        den = dp.tile([W, H], f32, tag="den")
                        nc.vector.tensor_scalar_add(den[:, :], ps[:, 64:64 + H],
                                                    1e-10)
                        inv = dp.tile([W, H], f32, tag="inv")
                        nc.vector.reciprocal(inv[:, :], den[:, :])
                        ot = dp.tile([W, 64], f32, tag="ot")
                        if H == 1:
                            o_ap = _sub(ot[:, :], 0, [[64, 1], [1, 64]])
                            s_ap = _sub(ps[:, :], 0, [[64, 1], [1, 64]])
                            i_ap = _sub(inv[:, :], 0, [[1, 1], [0, 64]])
                        else:
                            o_ap = _sub(ot[:, :], 0, [[64 // H, H], [1, 64 // H]])
                            s_ap = _sub(ps[:, :], 0, [[64 // H, H], [1, 64 // H]])
                            i_ap = _sub(inv[:, :], 0, [[1, H], [0, 64 // H]])
                        nc.vector.tensor_tensor(out=o_ap, in0=s_ap, in1=i_ap,
                                                op=Alu.mult)
                        if layer == 1:
                            nc.vector.tensor_tensor(out=ot[:, :], in0=ot[:, :],
                                                    in1=b1r[:, :], op=Alu.add)
                            ex = dp.tile([W, 64], f32, tag="ex")
                            nc.scalar.activation(out=ex[:, :], in_=ot[:, :],
                                                 func=Act.Exp)
                            nc.vector.tensor_scalar(
                                out=ex[:, :], in0=ex[:, :], scalar1=-1.0,
                                scalar2=0.0, op0=Alu.add, op1=Alu.min)
                            rl = dp.tile([W, 64], f32, tag="rl")
                            nc.vector.tensor_scalar_max(rl[:, :], ot[:, :], 0.0)
                            pq, pr = (s % 2) * W, (s // 2) * W
                            nc.vector.tensor_tensor(
                                out=h2big[pq:pq + W, pr:pr + W],
                                in0=ex[:, :], in1=rl[:, :], op=Alu.add)
                        else:
                            ob = dp.tile([W, 64], f32, tag="ob")
                            nc.vector.tensor_tensor(out=ob[:, :], in0=ot[:, :],
                                                    in1=b2r[:, :], op=Alu.add)
                            nc.sync.dma_start(
                                out=out_d[s * W:(s + 1) * W, :], in_=ob[:, :])

            edge_phase(table1, 80, 72, HEADS1, esrc, edst, layer=1)

            # ---------- node phase, layer 2 (from SBUF h2big) ----------
            for t in range(nt):
                tp2 = np_.tile([64, P], f32, tag="tps")
                nc.tensor.transpose(tp2[:, :], h2big[:, t * 64:(t + 1) * 64],
                                    ident[:, :])
                h2T = wp.tile([64, P], f32, tag="h2T")
                nc.vector.tensor_copy(out=h2T[:, :], in_=tp2[:, :])
                hp2 = np_.tile([P, 66], f32, tag="hps")
                nc.tensor.matmul(out=hp2[:, :], lhsT=h2T[:, :], rhs=wc2[:, :],
                                 start=True, stop=True)
                h2t = wp.tile([P, 66], f32, tag="ht")
                nc.vector.tensor_copy(out=h2t[:, :], in_=hp2[:, :])
                nc.sync.dma_start(out=t2s_d[t * P:(t + 1) * P, :], in_=h2t[:, :])

            nc.gpsimd.collective_compute(
                "AllGather", Alu.bypass, replica_groups=groups,
                ins=[t2s_d[:, :]], outs=[table2[:, :]])
            if debug_tabs:
                for t in range(n_pad // P):
                    dt2_ = wp.tile([P, 66], f32, tag="dbg")
                    nc.sync.dma_start(out=dt2_[:, :], in_=table2[t*P:(t+1)*P, :])
                    nc.sync.dma_start(out=dbg2_d[t*P:(t+1)*P, :], in_=dt2_[:, :])

            edge_phase(table2, 66, 65, 1, esrc2, edst2, layer=2)


def kernel(**inputs):
    import concourse.bacc as bacc
    from concourse.bass_utils import run_bass_kernel_spmd

    n = inputs["x"].shape[0]
    cfg, in_maps, perm = _prep(**inputs)

    nc = bacc.Bacc("TRN2", target_bir_lowering=False, debug=False,
                   num_devices=cfg["n_cores"])
    _build(nc, cfg)
    nc.compile()

    res = run_bass_kernel_spmd(nc, in_maps,
                               core_ids=list(range(cfg["n_cores"])))
    full = np.concatenate([r["out"] for r in res.results], axis=0)
    out = full[perm[:n]]
    return np.ascontiguousarray(out, np.float32)
